# revision 4
# baseline (speedup 1.0000x reference)
"""Trainium2 Bass kernel for the GRU decoder with categorical sampling.

Strategy (8 NeuronCores):
- Vocab projection W_out [V,H] sharded over cores: 4096 rows/core (core 7 padded),
  resident in SBUF. Logits computed feature-major, transposed to batch-major on PE.
- Sampling: z = logits + gumbel (gumbel precomputed on host from the fixed jax key,
  b_out folded in). Per-window top8 via vector.max/max_index, cross-core argmax via
  a tiny AllGather per step. z is written out; host reconstructs logits = z - gumbel.
- GRU replicated on all cores (no comms for h). W_hh streamed from HBM each step
  (SBUF can't hold both the W_out shard and W_hh). The input-side matmul is replaced
  by a gather from the host-precomputed fused table M_ie = emb @ W_ih.T + b_ih (+
  b_hh for r,z gates).
- Gathered gi rows (batch-major) are folded into the feature-major gate PSUM via
  accumulating PE transposes.
"""
import numpy as np

B = 64
V = 32000
E = 512
H = 1024
NCORES = 8
VSH = 4096          # per-core padded vocab shard
NW = 8              # z windows per step (VSH / 512)
GW = 256            # W_hh stream window (gates per window)
NGW = 3072 // GW    # number of W_hh stream windows (12)
KC = H // 128       # h k-chunks (8)


def _build(nc, T_steps):
    import concourse.bass as bass
    import concourse.mybir as mybir
    from concourse import tile
    from concourse.masks import make_identity

    f32 = mybir.dt.float32
    u32 = mybir.dt.uint32
    AF = mybir.ActivationFunctionType
    OP = mybir.AluOpType
    AX = mybir.AxisListType

    d_mie = nc.dram_tensor("mie", [V, 3072], f32, kind="ExternalInput")
    d_whh = nc.dram_tensor("whh", [NGW, 128, KC * GW], f32, kind="ExternalInput")
    d_wout = nc.dram_tensor("wout", [128, KC, VSH], f32, kind="ExternalInput")
    d_gum = nc.dram_tensor("gum", [T_steps, B, VSH], f32, kind="ExternalInput")
    d_h0 = nc.dram_tensor("h0t", [128, KC, B], f32, kind="ExternalInput")
    d_sid0 = nc.dram_tensor("sid0", [B, 1], u32, kind="ExternalInput")
    d_woff = nc.dram_tensor("woff", [B, NW], f32, kind="ExternalInput")
    d_bhhn = nc.dram_tensor("bhhn", [1, H], f32, kind="ExternalInput")

    d_z = nc.dram_tensor("z_out", [T_steps, B, VSH], f32, kind="ExternalOutput")
    d_sid = nc.dram_tensor("sid_out", [B, T_steps], f32, kind="ExternalOutput")

    with tile.TileContext(nc) as tc:
        with (
            tc.tile_pool(name="const", bufs=1) as cpool,
            tc.tile_pool(name="wout", bufs=1) as wout_pool,
            tc.tile_pool(name="whhw", bufs=2) as whh_pool,
            tc.tile_pool(name="gum", bufs=2) as gum_pool,
            tc.tile_pool(name="zw", bufs=2) as z_pool,
            tc.tile_pool(name="gi", bufs=1) as gi_pool,
            tc.tile_pool(name="h", bufs=2) as h_pool,
            tc.tile_pool(name="gate", bufs=1) as gate_pool,
            tc.tile_pool(name="samp", bufs=2) as samp_pool,
            tc.tile_pool(name="sid", bufs=2) as sid_pool,
            tc.tile_pool(name="ps_g", bufs=1, space="PSUM") as ps_g,
            tc.tile_pool(name="ps_v", bufs=2, space="PSUM") as ps_v,
            tc.tile_pool(name="ps_t", bufs=1, space="PSUM") as ps_t,
            tc.tile_pool(name="ps_m", bufs=1, space="PSUM") as ps_m,
            tc.tile_pool(name="dram", bufs=2, space="DRAM") as dram_pool,
        ):
            ident = cpool.tile([128, 128], f32)
            make_identity(nc, ident[:])
            ones1 = cpool.tile([1, B], f32)
            nc.vector.memset(ones1[:], 1.0)
            woff = cpool.tile([B, NW], f32)
            nc.sync.dma_start(woff[:], d_woff.ap())
            bhhn = cpool.tile([1, H], f32)
            nc.sync.dma_start(bhhn[:], d_bhhn.ap())
            wout_t = wout_pool.tile([128, KC, VSH], f32)
            nc.sync.dma_start(wout_t[:], d_wout.ap())
            sid_acc = cpool.tile([B, T_steps], f32)

            h_cur = h_pool.tile([128, KC, B], f32, tag="h")
            nc.sync.dma_start(h_cur[:], d_h0.ap())
            sid_cur = sid_pool.tile([B, 1], u32, tag="sid")
            nc.sync.dma_start(sid_cur[:], d_sid0.ap())

            for t in range(T_steps):
                # ---- gather gi = M_ie[sid] ----
                gi = gi_pool.tile([B, 3072], f32, tag="gi")
                nc.gpsimd.indirect_dma_start(
                    out=gi[:],
                    out_offset=None,
                    in_=d_mie.ap(),
                    in_offset=bass.IndirectOffsetOnAxis(ap=sid_cur[:, :1], axis=0),
                )

                # ---- gate psums ----
                prz0 = ps_g.tile([128, 8, B], f32, tag="prz0")  # r gates
                prz1 = ps_g.tile([128, 8, B], f32, tag="prz1")  # z gates
                pgn = ps_g.tile([128, 8, B], f32, tag="pgn")    # gh_n + b_hh_n
                pgin = ps_g.tile([128, 8, B], f32, tag="pgin")  # gi_n

                def gslice(c):
                    if c < 8:
                        return prz0[:, c, :]
                    if c < 16:
                        return prz1[:, c - 8, :]
                    return pgn[:, c - 16, :]

                for w in range(NGW):
                    wt = whh_pool.tile([128, KC, GW], f32, tag="whhw")
                    nc.sync.dma_start(
                        wt[:], d_whh.ap()[w].rearrange("p (k g) -> p k g", k=KC))
                    for cl in range(GW // 128):
                        c = w * (GW // 128) + cl
                        ps = gslice(c)
                        for k in range(KC):
                            nc.tensor.matmul(
                                ps, wt[:, k, cl * 128:(cl + 1) * 128], h_cur[:, k, :],
                                start=(k == 0), stop=False)
                        if c < 16:
                            # + gi^T closes the accumulation
                            nc.tensor.matmul(
                                ps, gi[:, c * 128:(c + 1) * 128], ident[:B, :B],
                                is_transpose=True, start=False, stop=True)
                        else:
                            # + b_hh_n (rank-1 ones-row matmul) closes it
                            nc.tensor.matmul(
                                ps, bhhn[0:1, (c - 16) * 128:(c - 15) * 128],
                                ones1[0:1, :], start=False, stop=True)
                # gi_n^T kept separate (multiplied by r before adding)
                for j in range(8):
                    c = 16 + j
                    nc.tensor.matmul(
                        pgin[:, j, :], gi[:, c * 128:(c + 1) * 128], ident[:B, :B],
                        is_transpose=True, start=True, stop=True)

                # ---- GRU elementwise (feature-major) ----
                rz = gate_pool.tile([128, 2, 8, B], f32, tag="rz")
                nc.scalar.activation(rz[:, 0], prz0[:], AF.Sigmoid)
                nc.scalar.activation(rz[:, 1], prz1[:], AF.Sigmoid)
                rn = gate_pool.tile([128, 8, B], f32, tag="rn")
                nc.vector.tensor_tensor(rn[:], rz[:, 0], pgn[:], OP.mult)
                nc.vector.tensor_tensor(rn[:], rn[:], pgin[:], OP.add)
                n_sb = gate_pool.tile([128, 8, B], f32, tag="n")
                nc.scalar.activation(n_sb[:], rn[:], AF.Tanh)
                h_new = h_pool.tile([128, KC, B], f32, tag="h")
                hmn = gate_pool.tile([128, 8, B], f32, tag="hmn")
                nc.vector.scalar_tensor_tensor(
                    hmn[:], in0=n_sb[:], scalar=-1.0, in1=h_cur[:],
                    op0=OP.mult, op1=OP.add)                       # h - n
                nc.vector.tensor_tensor(hmn[:], rz[:, 1], hmn[:], OP.mult)
                nc.vector.tensor_tensor(h_new[:], n_sb[:], hmn[:], OP.add)
                h_cur = h_new

                # ---- vocab projection + z + per-window top8 ----
                wmax = samp_pool.tile([B, NW, 8], f32, tag="wmax")
                widx = samp_pool.tile([B, NW, 8], u32, tag="widx")
                for w in range(NW):
                    pv = ps_v.tile([128, 4, B], f32, tag="pv")
                    for cl in range(4):
                        c = w * 4 + cl
                        for k in range(KC):
                            nc.tensor.matmul(
                                pv[:, cl, :],
                                wout_t[:, k, c * 128:(c + 1) * 128], h_cur[:, k, :],
                                start=(k == 0), stop=(k == KC - 1))
                    lv = samp_pool.tile([128, 4, B], f32, tag="lv")
                    nc.scalar.copy(lv[:], pv[:])
                    pt = ps_t.tile([B, 512], f32, tag="pt")
                    for cl in range(4):
                        nc.tensor.matmul(
                            pt[:, cl * 128:(cl + 1) * 128], lv[:, cl, :], ident[:],
                            is_transpose=True, start=True, stop=True)
                    gw = gum_pool.tile([B, 512], f32, tag="gum")
                    nc.sync.dma_start(gw[:], d_gum.ap()[t, :, w * 512:(w + 1) * 512])
                    zw = z_pool.tile([B, 512], f32, tag="zw")
                    nc.vector.tensor_tensor(zw[:], pt[:], gw[:], OP.add)
                    nc.sync.dma_start(d_z.ap()[t, :, w * 512:(w + 1) * 512], zw[:])
                    nc.vector.max(out=wmax[:, w, :], in_=zw[:])
                    nc.vector.max_index(out=widx[:, w, :], in_max=wmax[:, w, :],
                                        in_values=zw[:])

                # ---- local winner (value, global vocab index) ----
                lval = samp_pool.tile([B, 1], f32, tag="lval")
                nc.vector.tensor_reduce(lval[:], wmax[:, :, 0:1], axis=AX.XY, op=OP.max)
                idxf = samp_pool.tile([B, NW], f32, tag="idxf")
                nc.vector.tensor_copy(idxf[:], widx[:, :, 0])
                nc.vector.tensor_tensor(idxf[:], idxf[:], woff[:], OP.add)
                maskw = samp_pool.tile([B, NW], f32, tag="maskw")
                nc.vector.tensor_tensor(
                    maskw[:], wmax[:, :, 0], lval.to_broadcast([B, NW]), OP.is_ge)
                pen = samp_pool.tile([B, NW], f32, tag="pen")
                nc.vector.tensor_scalar(pen[:], maskw[:], -1e9, 1e9,
                                        op0=OP.mult, op1=OP.add)
                nc.vector.tensor_tensor(idxf[:], idxf[:], maskw[:], OP.mult)
                nc.vector.tensor_tensor(idxf[:], idxf[:], pen[:], OP.add)
                lidx = samp_pool.tile([B, 1], f32, tag="lidx")
                nc.vector.tensor_reduce(lidx[:], idxf[:], axis=AX.X, op=OP.min)

                # ---- cross-core argmax via AllGather ----
                cand = samp_pool.tile([B, 2], f32, tag="cand")
                nc.vector.tensor_copy(cand[:, 0:1], lval[:])
                nc.vector.tensor_copy(cand[:, 1:2], lidx[:])
                pc = ps_m.tile([2, B], f32, tag="pm")
                nc.tensor.matmul(pc[:], cand[:], ident[:B, :B],
                                 is_transpose=True, start=True, stop=True)
                candT = samp_pool.tile([2, B], f32, tag="candT")
                nc.vector.tensor_copy(candT[:], pc[:])
                agin = dram_pool.tile([2, B], f32, tag="agin")
                agout = dram_pool.tile([2 * NCORES, B], f32, tag="agout")
                nc.sync.dma_start(agin[:], candT[:])
                nc.gpsimd.collective_compute(
                    "AllGather", OP.bypass,
                    replica_groups=[list(range(NCORES))],
                    ins=[agin.opt()], outs=[agout.opt()])
                call = samp_pool.tile([2 * NCORES, B], f32, tag="call")
                nc.sync.dma_start(call[:], agout[:])
                pall = ps_m.tile([B, 2 * NCORES], f32, tag="pm")
                nc.tensor.matmul(pall[:], call[:], ident[:2 * NCORES, :2 * NCORES],
                                 is_transpose=True, start=True, stop=True)
                ca = samp_pool.tile([B, NCORES, 2], f32, tag="ca")
                nc.vector.tensor_copy(ca[:], pall[:])

                gmax = samp_pool.tile([B, 1], f32, tag="gmax")
                nc.vector.tensor_reduce(gmax[:], ca[:, :, 0:1], axis=AX.XY, op=OP.max)
                maskc = samp_pool.tile([B, NCORES], f32, tag="maskc")
                nc.vector.tensor_tensor(
                    maskc[:], ca[:, :, 0], gmax.to_broadcast([B, NCORES]), OP.is_ge)
                penc = samp_pool.tile([B, NCORES], f32, tag="penc")
                nc.vector.tensor_scalar(penc[:], maskc[:], -1e9, 1e9,
                                        op0=OP.mult, op1=OP.add)
                idxc = samp_pool.tile([B, NCORES], f32, tag="idxc")
                nc.vector.tensor_tensor(idxc[:], ca[:, :, 1], maskc[:], OP.mult)
                nc.vector.tensor_tensor(idxc[:], idxc[:], penc[:], OP.add)
                sid_f = samp_pool.tile([B, 1], f32, tag="sidf")
                nc.vector.tensor_reduce(sid_f[:], idxc[:], axis=AX.X, op=OP.min)
                nc.vector.tensor_copy(sid_acc[:, t:t + 1], sid_f[:])
                sid_new = sid_pool.tile([B, 1], u32, tag="sid")
                nc.vector.tensor_copy(sid_new[:], sid_f[:])
                sid_cur = sid_new

            nc.sync.dma_start(d_sid.ap(), sid_acc[:])
    return nc


def _host_prep(inputs, max_length, init_hidden, att_embedding, emb_table,
               W_ih, W_hh, b_ih, b_hh, W_out, b_out, T_steps):
    import jax
    import jax.numpy as jnp

    cpu = jax.devices("cpu")[0]
    inputs = np.asarray(inputs)
    init_hidden = np.asarray(init_hidden, np.float32)
    att_embedding = np.asarray(att_embedding, np.float32)
    emb_table = np.asarray(emb_table, np.float32)
    W_ih = np.asarray(W_ih, np.float32)
    W_hh = np.asarray(W_hh, np.float32)
    b_ih = np.asarray(b_ih, np.float32)
    b_hh = np.asarray(b_hh, np.float32)
    W_out = np.asarray(W_out, np.float32)
    b_out = np.asarray(b_out, np.float32)

    with jax.default_device(cpu):
        bias = b_ih + np.concatenate([b_hh[:2 * H], np.zeros(H, np.float32)])
        mie = np.asarray(jnp.asarray(emb_table) @ jnp.asarray(W_ih).T) + bias
        mie = np.ascontiguousarray(mie, np.float32)
        keys = jax.random.split(jax.random.key(42), int(max_length))[:T_steps]
        # NOTE: vmap(gumbel) is NOT bitwise-identical to per-key gumbel calls;
        # the reference samples per step, so generate per key.
        G = np.stack([
            np.asarray(jax.random.gumbel(keys[t], (B, V), jnp.float32))
            for t in range(T_steps)])

    whh_l = np.ascontiguousarray(
        W_hh.reshape(NGW, GW, KC, 128).transpose(0, 3, 2, 1).reshape(
            NGW, 128, KC * GW))

    h0 = np.concatenate([init_hidden, att_embedding], axis=2)[0]  # [B, H]
    h0t = np.ascontiguousarray(h0.T.reshape(KC, 128, B).transpose(1, 0, 2))
    sid0 = np.ascontiguousarray(inputs[:, 0:1].astype(np.uint32))
    bhhn = np.ascontiguousarray(b_hh[2 * H:].reshape(1, H))

    in_maps, G_shards = [], []
    for c in range(NCORES):
        lo = c * VSH
        hi = min(V, lo + VSH)
        wsh = np.zeros((VSH, H), np.float32)
        wsh[: hi - lo] = W_out[lo:hi]
        wout_l = np.ascontiguousarray(wsh.T.reshape(KC, 128, VSH).transpose(1, 0, 2))
        gsh = np.full((T_steps, B, VSH), -1e30, np.float32)
        gsh[:, :, : hi - lo] = G[:, :, lo:hi] + b_out[lo:hi]
        G_shards.append(np.ascontiguousarray(G[:, :, lo:hi]))
        woff = np.ascontiguousarray(np.broadcast_to(
            (lo + 512.0 * np.arange(NW, dtype=np.float32))[None, :], (B, NW)))
        in_maps.append(dict(
            mie=mie, whh=whh_l, wout=wout_l, gum=np.ascontiguousarray(gsh),
            h0t=h0t, sid0=sid0, woff=woff, bhhn=bhhn))
    return in_maps, G_shards


def kernel(inputs, max_length, init_hidden, att_embedding, emb_table,
           W_ih, W_hh, b_ih, b_hh, W_out, b_out, _T_steps=None, _trace=False):
    import concourse.bacc as bacc
    from concourse.bass_utils import run_bass_kernel_spmd

    T_steps = int(_T_steps if _T_steps is not None else max_length)

    in_maps, G_shards = _host_prep(
        inputs, max_length, init_hidden, att_embedding, emb_table,
        W_ih, W_hh, b_ih, b_hh, W_out, b_out, T_steps)

    nc = bacc.Bacc(None, target_bir_lowering=False, debug=False,
                   num_devices=NCORES)
    _build(nc, T_steps)
    nc.compile()

    res = run_bass_kernel_spmd(nc, in_maps, core_ids=list(range(NCORES)),
                               trace=_trace)
    results = res.results

    logits = np.empty((T_steps, B, V), np.float32)
    for c in range(NCORES):
        lo = c * VSH
        hi = min(V, lo + VSH)
        z = results[c]["z_out"][:, :, : hi - lo]
        logits[:, :, lo:hi] = (
            z.astype(np.float64)
            - G_shards[c][:, :, : hi - lo].astype(np.float64)).astype(np.float32)
    sampled = results[0]["sid_out"].astype(np.int32)  # [B, T]
    if _trace and res.exec_time_ns:
        print("HW exec time:", res.exec_time_ns, "ns")
    return logits.reshape(T_steps * B, V), sampled
